# revision 34
# baseline (speedup 1.0000x reference)
"""Trainium2 Bass kernel for nn_DNN_24464133718540 (embedding_lookup).

Reference computation:
    emb[b,f]  = tables[f, src[b,f]]            # [B, 45, 256] gather
    h         = emb @ W1 + b1                  # [B, 45, 32]
    out[b,f]  = h @ W2 + b2                    # [B, 45, 1]
    result[b] = sum_f out[b,f]                 # [B, 1]

The MLP is linear (no activation), so with w = W1 @ W2 ([256]) and
c = b1 @ W2 + b2 (scalar):
    result[b] = sum_f tables[f, src[b,f]] . w  +  45 * c

Host staging folds w into the tables elementwise (tab*w, cast to bf16)
so the device only row-sums and gathers; the 45 tables are viewed as one
[450000, 256] row space split uniformly across the 8 cores (56250 rows
each = 6 slots x 9375 rows, so every core streams the same 28.8 MB).

Device kernel per slot (SPMD over 8 cores):
  phase 1: stream the slot's 9472 rows (97 zero-pad) from HBM in two
           2.4 MB chunks ([p=128, j=37 rows, d=256] bf16) split over the
           two HW-DGE rings (each ring executes its DMAs serially with a
           ~2.3us per-DMA overhead, so big chunks + few small DMAs);
           DVE tree-adds halves (2x packed bf16) then tensor_reduce ->
           scores cols [128, 74] fp32.
  phase 2: PE transpose -> PSUM [74,128]; DVE evac; flatten to a score
           row [1, 9472]; log-replicate (1,2,4 partitions) to the 8
           gather-group base partitions; ap_gather pulls the scores of
           the slot's DISTINCT referenced rows (<= 7680; Q7 ap_gather
           time is the bottleneck so dedup halves it). The gpsimd queue
           carries ONLY ap_gathers (any SWDGE DMA there would force a
           Q7 library reload around every gather); outputs accumulate in
           one SBUF tile, written once by a single final DMA.
Host: scatter-add the gathered distinct-row scores into [B] via the
unique-inverse maps, add 45*c.
"""

import numpy as np
import ml_dtypes

B, F, V, D, H = 16384, 45, 10000, 256, 32
NCORES = 8
NSLOT = 6                    # virtual tables per core
RPS = 9375                   # rows per slot (45*10000 / 48)
CJ = 16                      # rows per partition per chunk
CROWS = 128 * CJ             # 2048 rows per full chunk
NCHUNK = 5                   # 4 full chunks + 1 partial (p=74): 1 MB chunks
RPS_PAD = 9376               # staged rows (1 zero-pad row)
VCH = NCHUNK * CJ            # 80 score columns per slot
VPAD = VCH * 128             # 10240 flattened score-row length
NBLK = 8                     # gather blocks (Q7 groups)
BLK = 960                    # gathered values per block (dedup: 7680/slot)
GSLOT = NBLK * BLK           # 7680 gather slots per table-slot
SROW = BLK // 16             # 60 int16 idx entries per channel per slot
PAD_ROW = RPS                # 9375: a zero pad row

_COMPILED = {}


def _pos_of_local(local):
    """flattened score-row position for local row index (vectorized)."""
    c = local // CROWS
    r = local % CROWS
    return (c * CJ + r % CJ) * 128 + r // CJ


PAD_POS = int(_pos_of_local(np.int64(PAD_ROW)))   # zero score position


def _build_program():
    import concourse.bacc as bacc
    import concourse.tile as tile
    from concourse import mybir

    f32 = mybir.dt.float32
    bf16 = mybir.dt.bfloat16
    i16 = mybir.dt.int16

    nc = bacc.Bacc("TRN2", target_bir_lowering=False, debug=False,
                   num_devices=NCORES)

    tables_c = nc.dram_tensor("tables_c", [NSLOT, RPS_PAD, D], bf16,
                              kind="ExternalInput")
    ident_d = nc.dram_tensor("ident", [128, 128], f32, kind="ExternalInput")
    # idx staged host-side in the on-chip layout: one contiguous DMA (the
    # old [f p s]->[p f s] rearrange emitted 768 tiny descriptors, delaying
    # the first stream chunks behind it on the sync ring)
    idx_d = nc.dram_tensor("idx16", [128, NSLOT * SROW], i16,
                           kind="ExternalInput")
    out_d = nc.dram_tensor("out_part", [NSLOT, NBLK, BLK], f32,
                           kind="ExternalOutput")

    with tile.TileContext(nc) as tc:
        with (
            tc.tile_pool(name="const", bufs=1) as const_pool,
            tc.tile_pool(name="stream", bufs=6) as stream_pool,
            tc.tile_pool(name="tree", bufs=2) as tree_pool,
            tc.tile_pool(name="cols", bufs=2) as cols_pool,
            tc.tile_pool(name="row", bufs=2) as row_pool,
            tc.tile_pool(name="rep", bufs=2) as rep_pool,
            tc.tile_pool(name="pst", bufs=2, space="PSUM") as psum_t_pool,
        ):
            # one-time constants
            ident_t = const_pool.tile([128, 128], f32, tag="ident")
            nc.scalar.dma_start(ident_t[:], ident_d.ap())
            idx_t = const_pool.tile([128, NSLOT * SROW], i16, tag="idx")
            nc.scalar.dma_start(idx_t[:], idx_d.ap())
            gout_all = const_pool.tile([128, NSLOT * BLK], f32, tag="gout")

            # dummy 16-idx gather at t~0 forces the Q7 ap_gather library to
            # load while slot 0 streams (lazy load costs ~10us serial)
            zidx = const_pool.tile([128, 1], i16, tag="zidx")
            nc.vector.memset(zidx[:], 0)
            warm = const_pool.tile([128, 16], f32, tag="warm")
            winst = nc.gpsimd.ap_gather(
                out_ap=warm[:], in_ap=ident_t[:], idxs_ap=zidx[:],
                channels=128, num_elems=128, d=1, num_idxs=16)
            winst.verify = False

            tab_ap = tables_c.ap()  # [NSLOT, RPS_PAD, D]

            rep_hold = {}

            def phase2a(f, cols):
                # transpose -> PSUM evac -> flatten row into rep partition 0,
                # then log-replicate p0 -> p{16,32,...,112} (1, 2, then 4
                # strided partitions per DMA). All smalls ride one ring
                # (alternating by slot) at the slot's ring tail.
                ring = nc.scalar if f % 2 == 0 else nc.sync
                pt = psum_t_pool.tile([VCH, 128], f32, tag="pt")
                nc.tensor.transpose(pt[:], cols[:, :VCH], ident_t[:])
                ptsb = row_pool.tile([VCH, 128], f32, tag="ptsb")
                nc.vector.tensor_copy(ptsb[:], pt[:])
                rep = rep_pool.tile([128, VPAD], f32, tag="rep")
                ring.dma_start(
                    rep[0:1, :].rearrange("o (c p) -> o c p", c=VCH), ptsb[:])
                repv = rep[:].rearrange("(a g) n -> a g n", g=16)
                ring.dma_start(repv[1:2, 0], repv[0:1, 0])
                ring.dma_start(repv[2:4, 0], repv[0:2, 0])
                ring.dma_start(repv[4:8, 0], repv[0:4, 0])
                rep_hold[f] = rep

            def phase2b(f):
                # gpsimd runs ONLY ap_gathers; results accumulate in gout_all.
                rep = rep_hold.pop(f)
                inst = nc.gpsimd.ap_gather(
                    out_ap=gout_all[:, f * BLK:(f + 1) * BLK],
                    in_ap=rep[:, :VPAD],
                    idxs_ap=idx_t[:, f * SROW:(f + 1) * SROW],
                    channels=128,
                    num_elems=VPAD,
                    d=1,
                    num_idxs=BLK,
                )
                # skip the ucode's per-index bounds-verify pass (indices are
                # host-validated; they are always in [0, VPAD))
                inst.verify = False

            for f in range(NSLOT):
                cols = cols_pool.tile([128, VCH], f32, tag="cols")
                nc.vector.memset(cols[:], 0.0)
                # ---- phase 1: stream + tree-reduce (w pre-folded on host) --
                for c in range(NCHUNK):
                    p = 128 if c < NCHUNK - 1 else (RPS_PAD - 4 * CROWS) // CJ
                    st = stream_pool.tile([128, CJ * D], bf16, tag="st")
                    src_ap = tab_ap[f, c * CROWS:c * CROWS + p * CJ, :].rearrange(
                        "(p j) d -> p j d", p=p)
                    ring = nc.sync if (f * NCHUNK + c) % 2 == 0 else nc.scalar
                    ring.dma_start(st[:p], src_ap)
                    # bf16 pairwise adds run in 2x packed mode; the final
                    # 16-wide tensor_reduce emits fp32 score columns.
                    stv = st[:p].rearrange("p (j h d) -> p j h d", j=CJ, h=2)
                    s1 = tree_pool.tile([128, CJ * 128], bf16, tag="s1")
                    nc.vector.tensor_add(
                        s1[:p].rearrange("p (j d) -> p j d", j=CJ),
                        stv[:, :, 0], stv[:, :, 1])
                    s1v = s1[:p].rearrange("p (j h d) -> p j h d", j=CJ, h=2)
                    s2 = tree_pool.tile([128, CJ * 64], bf16, tag="s2")
                    nc.vector.tensor_add(
                        s2[:p].rearrange("p (j d) -> p j d", j=CJ),
                        s1v[:, :, 0], s1v[:, :, 1])
                    s2v = s2[:p].rearrange("p (j h d) -> p j h d", j=CJ, h=2)
                    s3 = tree_pool.tile([128, CJ * 32], bf16, tag="s3")
                    nc.vector.tensor_add(
                        s3[:p].rearrange("p (j d) -> p j d", j=CJ),
                        s2v[:, :, 0], s2v[:, :, 1])
                    s3v = s3[:p].rearrange("p (j h d) -> p j h d", j=CJ, h=2)
                    s4 = tree_pool.tile([128, CJ * 16], bf16, tag="s4")
                    nc.vector.tensor_add(
                        s4[:p].rearrange("p (j d) -> p j d", j=CJ),
                        s3v[:, :, 0], s3v[:, :, 1])
                    nc.vector.tensor_reduce(
                        cols[:p, c * CJ:(c + 1) * CJ],
                        s4[:p].rearrange("p (j d) -> p j d", j=CJ),
                        axis=mybir.AxisListType.X,
                        op=mybir.AluOpType.add,
                    )

                phase2a(f, cols)
                phase2b(f)

            # single final output DMA: [8 group-base partitions, 6*960]
            nc.scalar.dma_start(
                out_d.ap().rearrange("f k n -> k f n"),
                gout_all[:].rearrange("(k g) (f n) -> k g f n", g=16, f=NSLOT
                                      )[:, 0])

    nc.compile()
    return nc


def _get_program():
    if "nc" not in _COMPILED:
        _COMPILED["nc"] = _build_program()
    return _COMPILED["nc"]


def kernel(src, tables, W1, b1, W2, b2, _trace=False, _trace_cores=None,
           _tmpdir=None):
    from concourse.bass_utils import run_bass_kernel_spmd

    src = np.asarray(src)
    tables = np.asarray(tables, dtype=np.float32)
    W1 = np.asarray(W1, dtype=np.float32)
    b1 = np.asarray(b1, dtype=np.float32)
    W2 = np.asarray(W2, dtype=np.float32)
    b2 = np.asarray(b2, dtype=np.float32)

    w = (W1 @ W2).reshape(D)                      # [256]
    c = float(b1 @ W2[:, 0] + b2[0])              # scalar per feature

    # fold w into the tables and cast to bf16; flatten to one row space
    flat = (tables.reshape(F * V, D) * w[None, :]).astype(ml_dtypes.bfloat16)

    RPC = NSLOT * RPS                             # 56250 rows per core
    src_i = np.asarray(src, dtype=np.int64)
    g = (np.arange(F, dtype=np.int64)[None, :] * V + src_i).ravel()   # [B*F]
    b_of = np.broadcast_to(
        np.arange(B, dtype=np.int32)[:, None], (B, F)).ravel()
    core_of = g // RPC
    slot_of = (g % RPC) // RPS
    local = (g % RPC) % RPS

    in_maps = []
    assembly = []   # per core per slot: (b_refs, inverse, spill)
    for core in range(NCORES):
        tc_arr = np.zeros((NSLOT, RPS_PAD, D), dtype=ml_dtypes.bfloat16)
        rows = flat[core * RPC:(core + 1) * RPC].reshape(NSLOT, RPS, D)
        tc_arr[:, :RPS, :] = rows

        idx16 = np.zeros((NSLOT, 128, SROW), dtype=np.int16)   # pre-transpose
        per_slot = []
        m_core = core_of == core
        for s in range(NSLOT):
            m = m_core & (slot_of == s)
            locs = local[m]
            bs = b_of[m]
            rows_u, inv = np.unique(locs, return_inverse=True)
            n_u = rows_u.shape[0]
            spill = None
            if n_u > GSLOT:
                # rows beyond device gather capacity: host-summed (rare)
                sp_rows = rows_u[GSLOT:]
                keep = inv < GSLOT
                spill = (sp_rows, bs[~keep], inv[~keep] - GSLOT)
                bs, inv = bs[keep], inv[keep]
                rows_u, n_u = rows_u[:GSLOT], GSLOT
            full = np.full(GSLOT, PAD_POS, dtype=np.int16)
            full[:n_u] = _pos_of_local(rows_u.astype(np.int64)).astype(np.int16)
            # idx16[s, 16k+p, t] = pos of distinct-row number BLK*k + 16t + p
            idx16[s] = (full.reshape(NBLK, SROW, 16)
                        .transpose(0, 2, 1)
                        .reshape(128, SROW))
            per_slot.append((bs, inv, spill))
        assembly.append(per_slot)
        in_maps.append({
            "tables_c": tc_arr,
            "ident": np.eye(128, dtype=np.float32),
            # [128, NSLOT*SROW]: the device's on-chip idx layout
            "idx16": np.ascontiguousarray(
                idx16.transpose(1, 0, 2).reshape(128, NSLOT * SROW)),
        })

    nc = _get_program()
    kw = {}
    if _trace:
        kw = {"trace": True, "trace_cores": _trace_cores or [0],
              "tmpdir": _tmpdir}
    res = run_bass_kernel_spmd(nc, in_maps, core_ids=list(range(NCORES)), **kw)
    _COMPILED["last_results"] = res

    total = np.zeros(B, dtype=np.float64)
    for core in range(NCORES):
        vals = res.results[core]["out_part"].reshape(NSLOT, GSLOT)
        for s in range(NSLOT):
            bs, inv, spill = assembly[core][s]
            np.add.at(total, bs, vals[s][inv].astype(np.float64))
            if spill is not None:
                sp_rows, sp_b, sp_inv = spill
                base = core * RPC + s * RPS
                sp_scores = flat[base + sp_rows].astype(np.float64).sum(axis=1)
                np.add.at(total, sp_b, sp_scores[sp_inv])
    total += F * c
    return total.astype(np.float32).reshape(B, 1)


# revision 35
# speedup vs baseline: 1.1079x; 1.1079x over previous
"""Trainium2 Bass kernel for nn_DNN_24464133718540 (embedding_lookup).

Reference computation:
    emb[b,f]  = tables[f, src[b,f]]            # [B, 45, 256] gather
    h         = emb @ W1 + b1                  # [B, 45, 32]
    out[b,f]  = h @ W2 + b2                    # [B, 45, 1]
    result[b] = sum_f out[b,f]                 # [B, 1]

The MLP is linear (no activation), so with w = W1 @ W2 ([256]) and
c = b1 @ W2 + b2 (scalar):
    result[b] = sum_f tables[f, src[b,f]] . w  +  45 * c

Host staging folds w into the tables elementwise (tab*w, cast to bf16)
so the device only row-sums and gathers; the 45 tables are viewed as one
[450000, 256] row space split uniformly across the 8 cores (56250 rows
each = 6 slots x 9375 rows, so every core streams the same 28.8 MB).

Device kernel per slot (SPMD over 8 cores):
  phase 1: stream the slot's 9472 rows (97 zero-pad) from HBM in two
           2.4 MB chunks ([p=128, j=37 rows, d=256] bf16) split over the
           two HW-DGE rings (each ring executes its DMAs serially with a
           ~2.3us per-DMA overhead, so big chunks + few small DMAs);
           DVE tree-adds halves (2x packed bf16) then tensor_reduce ->
           scores cols [128, 74] fp32.
  phase 2: PE transpose -> PSUM [74,128]; DVE evac; flatten to a score
           row [1, 9472]; log-replicate (1,2,4 partitions) to the 8
           gather-group base partitions; ap_gather pulls the scores of
           the slot's DISTINCT referenced rows (<= 7680; Q7 ap_gather
           time is the bottleneck so dedup halves it). The gpsimd queue
           carries ONLY ap_gathers (any SWDGE DMA there would force a
           Q7 library reload around every gather); outputs accumulate in
           one SBUF tile, written once by a single final DMA.
Host: scatter-add the gathered distinct-row scores into [B] via the
unique-inverse maps, add 45*c.
"""

import numpy as np
import ml_dtypes

B, F, V, D, H = 16384, 45, 10000, 256, 32
NCORES = 8
NSLOT = 6                    # virtual tables per core
RPS = 9375                   # rows per slot (45*10000 / 48)
CJ = 37                      # rows per partition per chunk
CROWS = 128 * CJ             # 4736 rows per chunk
NCHUNK = 2                   # two full chunks per slot
RPS_PAD = NCHUNK * CROWS     # 9472 staged rows (97 zero-pad)
VCH = NCHUNK * CJ            # 74 score columns per slot
VPAD = VCH * 128             # 9472 flattened score-row length
NBLK = 8                     # gather blocks (Q7 groups)
BLK = 960                    # gathered values per block (dedup: 7680/slot)
GSLOT = NBLK * BLK           # 7680 gather slots per table-slot
SROW = BLK // 16             # 60 int16 idx entries per channel per slot
PAD_ROW = RPS                # 9375: a zero pad row

_COMPILED = {}


def _pos_of_local(local):
    """flattened score-row position for local row index (vectorized)."""
    c = local // CROWS
    r = local % CROWS
    return (c * CJ + r % CJ) * 128 + r // CJ


PAD_POS = int(_pos_of_local(np.int64(PAD_ROW)))   # zero score position


def _build_program():
    import concourse.bacc as bacc
    import concourse.tile as tile
    from concourse import mybir

    f32 = mybir.dt.float32
    bf16 = mybir.dt.bfloat16
    i16 = mybir.dt.int16

    nc = bacc.Bacc("TRN2", target_bir_lowering=False, debug=False,
                   num_devices=NCORES)

    tables_c = nc.dram_tensor("tables_c", [NSLOT, RPS_PAD, D], bf16,
                              kind="ExternalInput")
    ident_d = nc.dram_tensor("ident", [128, 128], f32, kind="ExternalInput")
    # idx staged host-side in the on-chip layout: one contiguous DMA (the
    # old [f p s]->[p f s] rearrange emitted 768 tiny descriptors, delaying
    # the first stream chunks behind it on the sync ring)
    idx_d = nc.dram_tensor("idx16", [128, NSLOT * SROW], i16,
                           kind="ExternalInput")
    out_d = nc.dram_tensor("out_part", [NSLOT, NBLK, BLK], f32,
                           kind="ExternalOutput")

    with tile.TileContext(nc) as tc:
        with (
            tc.tile_pool(name="const", bufs=1) as const_pool,
            tc.tile_pool(name="stream", bufs=3) as stream_pool,
            tc.tile_pool(name="tree", bufs=2) as tree_pool,
            tc.tile_pool(name="cols", bufs=2) as cols_pool,
            tc.tile_pool(name="row", bufs=2) as row_pool,
            tc.tile_pool(name="rep", bufs=2) as rep_pool,
            tc.tile_pool(name="pst", bufs=2, space="PSUM") as psum_t_pool,
        ):
            # one-time constants
            ident_t = const_pool.tile([128, 128], f32, tag="ident")
            nc.scalar.dma_start(ident_t[:], ident_d.ap())
            idx_t = const_pool.tile([128, NSLOT * SROW], i16, tag="idx")
            nc.scalar.dma_start(idx_t[:], idx_d.ap())
            gout_all = const_pool.tile([128, NSLOT * BLK], f32, tag="gout")

            # dummy 16-idx gather at t~0 forces the Q7 ap_gather library to
            # load while slot 0 streams (lazy load costs ~10us serial)
            zidx = const_pool.tile([128, 1], i16, tag="zidx")
            nc.vector.memset(zidx[:], 0)
            warm = const_pool.tile([128, 16], f32, tag="warm")
            winst = nc.gpsimd.ap_gather(
                out_ap=warm[:], in_ap=ident_t[:], idxs_ap=zidx[:],
                channels=128, num_elems=128, d=1, num_idxs=16)
            winst.verify = False

            tab_ap = tables_c.ap()  # [NSLOT, RPS_PAD, D]

            rep_hold = {}

            def phase2a(f, cols):
                # transpose -> PSUM evac -> flatten row into rep partition 0,
                # then log-replicate p0 -> p{16,32,...,112} (1, 2, then 4
                # strided partitions per DMA). All smalls ride one ring
                # (alternating by slot) at the slot's ring tail.
                ring = nc.scalar if f % 2 == 0 else nc.sync
                pt = psum_t_pool.tile([VCH, 128], f32, tag="pt")
                nc.tensor.transpose(pt[:], cols[:, :VCH], ident_t[:])
                ptsb = row_pool.tile([VCH, 128], f32, tag="ptsb")
                nc.vector.tensor_copy(ptsb[:], pt[:])
                rep = rep_pool.tile([128, VPAD], f32, tag="rep")
                ring.dma_start(
                    rep[0:1, :].rearrange("o (c p) -> o c p", c=VCH), ptsb[:])
                repv = rep[:].rearrange("(a g) n -> a g n", g=16)
                ring.dma_start(repv[1:2, 0], repv[0:1, 0])
                ring.dma_start(repv[2:4, 0], repv[0:2, 0])
                ring.dma_start(repv[4:8, 0], repv[0:4, 0])
                rep_hold[f] = rep

            def phase2b(f):
                # gpsimd runs ONLY ap_gathers; results accumulate in gout_all.
                rep = rep_hold.pop(f)
                inst = nc.gpsimd.ap_gather(
                    out_ap=gout_all[:, f * BLK:(f + 1) * BLK],
                    in_ap=rep[:, :VPAD],
                    idxs_ap=idx_t[:, f * SROW:(f + 1) * SROW],
                    channels=128,
                    num_elems=VPAD,
                    d=1,
                    num_idxs=BLK,
                )
                # skip the ucode's per-index bounds-verify pass (indices are
                # host-validated; they are always in [0, VPAD))
                inst.verify = False

            for f in range(NSLOT):
                cols = cols_pool.tile([128, VCH], f32, tag="cols")
                nc.vector.memset(cols[:], 0.0)
                # ---- phase 1: stream + tree-reduce (w pre-folded on host) --
                for c in range(NCHUNK):
                    st = stream_pool.tile([128, CJ * D], bf16, tag="st")
                    src_ap = tab_ap[f, c * CROWS:(c + 1) * CROWS, :].rearrange(
                        "(p j) d -> p j d", p=128)
                    ring = nc.sync if (f + c) % 2 == 0 else nc.scalar
                    ring.dma_start(st[:], src_ap)
                    # bf16 pairwise adds run in 2x packed mode; the final
                    # 16-wide tensor_reduce emits fp32 score columns.
                    stv = st[:].rearrange("p (j h d) -> p j h d", j=CJ, h=2)
                    s1 = tree_pool.tile([128, CJ * 128], bf16, tag="s1")
                    nc.vector.tensor_add(
                        s1[:].rearrange("p (j d) -> p j d", j=CJ),
                        stv[:, :, 0], stv[:, :, 1])
                    s1v = s1[:].rearrange("p (j h d) -> p j h d", j=CJ, h=2)
                    s2 = tree_pool.tile([128, CJ * 64], bf16, tag="s2")
                    nc.vector.tensor_add(
                        s2[:].rearrange("p (j d) -> p j d", j=CJ),
                        s1v[:, :, 0], s1v[:, :, 1])
                    s2v = s2[:].rearrange("p (j h d) -> p j h d", j=CJ, h=2)
                    s3 = tree_pool.tile([128, CJ * 32], bf16, tag="s3")
                    nc.vector.tensor_add(
                        s3[:].rearrange("p (j d) -> p j d", j=CJ),
                        s2v[:, :, 0], s2v[:, :, 1])
                    s3v = s3[:].rearrange("p (j h d) -> p j h d", j=CJ, h=2)
                    s4 = tree_pool.tile([128, CJ * 16], bf16, tag="s4")
                    nc.vector.tensor_add(
                        s4[:].rearrange("p (j d) -> p j d", j=CJ),
                        s3v[:, :, 0], s3v[:, :, 1])
                    nc.vector.tensor_reduce(
                        cols[:, c * CJ:(c + 1) * CJ],
                        s4[:].rearrange("p (j d) -> p j d", j=CJ),
                        axis=mybir.AxisListType.X,
                        op=mybir.AluOpType.add,
                    )

                phase2a(f, cols)
                phase2b(f)

            # single final output DMA: [8 group-base partitions, 6*960]
            nc.scalar.dma_start(
                out_d.ap().rearrange("f k n -> k f n"),
                gout_all[:].rearrange("(k g) (f n) -> k g f n", g=16, f=NSLOT
                                      )[:, 0])

    nc.compile()
    return nc


def _get_program():
    if "nc" not in _COMPILED:
        _COMPILED["nc"] = _build_program()
    return _COMPILED["nc"]


def kernel(src, tables, W1, b1, W2, b2, _trace=False, _trace_cores=None,
           _tmpdir=None):
    from concourse.bass_utils import run_bass_kernel_spmd

    src = np.asarray(src)
    tables = np.asarray(tables, dtype=np.float32)
    W1 = np.asarray(W1, dtype=np.float32)
    b1 = np.asarray(b1, dtype=np.float32)
    W2 = np.asarray(W2, dtype=np.float32)
    b2 = np.asarray(b2, dtype=np.float32)

    w = (W1 @ W2).reshape(D)                      # [256]
    c = float(b1 @ W2[:, 0] + b2[0])              # scalar per feature

    # fold w into the tables and cast to bf16; flatten to one row space
    flat = (tables.reshape(F * V, D) * w[None, :]).astype(ml_dtypes.bfloat16)

    RPC = NSLOT * RPS                             # 56250 rows per core
    src_i = np.asarray(src, dtype=np.int64)
    g = (np.arange(F, dtype=np.int64)[None, :] * V + src_i).ravel()   # [B*F]
    b_of = np.broadcast_to(
        np.arange(B, dtype=np.int32)[:, None], (B, F)).ravel()
    core_of = g // RPC
    slot_of = (g % RPC) // RPS
    local = (g % RPC) % RPS

    in_maps = []
    assembly = []   # per core per slot: (b_refs, inverse, spill)
    for core in range(NCORES):
        tc_arr = np.zeros((NSLOT, RPS_PAD, D), dtype=ml_dtypes.bfloat16)
        rows = flat[core * RPC:(core + 1) * RPC].reshape(NSLOT, RPS, D)
        tc_arr[:, :RPS, :] = rows

        idx16 = np.zeros((NSLOT, 128, SROW), dtype=np.int16)   # pre-transpose
        per_slot = []
        m_core = core_of == core
        for s in range(NSLOT):
            m = m_core & (slot_of == s)
            locs = local[m]
            bs = b_of[m]
            rows_u, inv = np.unique(locs, return_inverse=True)
            n_u = rows_u.shape[0]
            spill = None
            if n_u > GSLOT:
                # rows beyond device gather capacity: host-summed (rare)
                sp_rows = rows_u[GSLOT:]
                keep = inv < GSLOT
                spill = (sp_rows, bs[~keep], inv[~keep] - GSLOT)
                bs, inv = bs[keep], inv[keep]
                rows_u, n_u = rows_u[:GSLOT], GSLOT
            full = np.full(GSLOT, PAD_POS, dtype=np.int16)
            full[:n_u] = _pos_of_local(rows_u.astype(np.int64)).astype(np.int16)
            # idx16[s, 16k+p, t] = pos of distinct-row number BLK*k + 16t + p
            idx16[s] = (full.reshape(NBLK, SROW, 16)
                        .transpose(0, 2, 1)
                        .reshape(128, SROW))
            per_slot.append((bs, inv, spill))
        assembly.append(per_slot)
        in_maps.append({
            "tables_c": tc_arr,
            "ident": np.eye(128, dtype=np.float32),
            # [128, NSLOT*SROW]: the device's on-chip idx layout
            "idx16": np.ascontiguousarray(
                idx16.transpose(1, 0, 2).reshape(128, NSLOT * SROW)),
        })

    nc = _get_program()
    kw = {}
    if _trace:
        kw = {"trace": True, "trace_cores": _trace_cores or [0],
              "tmpdir": _tmpdir}
    res = run_bass_kernel_spmd(nc, in_maps, core_ids=list(range(NCORES)), **kw)
    _COMPILED["last_results"] = res

    total = np.zeros(B, dtype=np.float64)
    for core in range(NCORES):
        vals = res.results[core]["out_part"].reshape(NSLOT, GSLOT)
        for s in range(NSLOT):
            bs, inv, spill = assembly[core][s]
            np.add.at(total, bs, vals[s][inv].astype(np.float64))
            if spill is not None:
                sp_rows, sp_b, sp_inv = spill
                base = core * RPC + s * RPS
                sp_scores = flat[base + sp_rows].astype(np.float64).sum(axis=1)
                np.add.at(total, sp_b, sp_scores[sp_inv])
    total += F * c
    return total.astype(np.float32).reshape(B, 1)


# revision 39
# speedup vs baseline: 1.1102x; 1.0020x over previous
"""Trainium2 Bass kernel for nn_DNN_24464133718540 (embedding_lookup).

Reference computation:
    emb[b,f]  = tables[f, src[b,f]]            # [B, 45, 256] gather
    h         = emb @ W1 + b1                  # [B, 45, 32]
    out[b,f]  = h @ W2 + b2                    # [B, 45, 1]
    result[b] = sum_f out[b,f]                 # [B, 1]

The MLP is linear (no activation), so with w = W1 @ W2 ([256]) and
c = b1 @ W2 + b2 (scalar):
    result[b] = sum_f tables[f, src[b,f]] . w  +  45 * c

Host staging folds w into the tables elementwise (tab*w, cast to bf16)
so the device only row-sums and gathers; the 45 tables are viewed as one
[450000, 256] row space split uniformly across the 8 cores (56250 rows
each = 6 slots x 9375 rows, so every core streams the same 28.8 MB).

Device kernel per slot (SPMD over 8 cores):
  phase 1: stream the slot's 9472 rows (97 zero-pad) from HBM in two
           2.4 MB chunks ([p=128, j=37 rows, d=256] bf16) split over the
           two HW-DGE rings (each ring executes its DMAs serially with a
           ~2.3us per-DMA overhead, so big chunks + few small DMAs);
           DVE tree-adds halves (2x packed bf16) then tensor_reduce ->
           scores cols [128, 74] fp32.
  phase 2: PE transpose -> PSUM [74,128]; DVE evac; flatten to a score
           row [1, 9472]; log-replicate (1,2,4 partitions) to the 8
           gather-group base partitions; ap_gather pulls the scores of
           the slot's DISTINCT referenced rows (<= 7680; Q7 ap_gather
           time is the bottleneck so dedup halves it). The gpsimd queue
           carries ONLY ap_gathers (any SWDGE DMA there would force a
           Q7 library reload around every gather); outputs accumulate in
           one SBUF tile, written once by a single final DMA.
Host: scatter-add the gathered distinct-row scores into [B] via the
unique-inverse maps, add 45*c.
"""

import numpy as np
import ml_dtypes

B, F, V, D, H = 16384, 45, 10000, 256, 32
NCORES = 8
NSLOT = 6                    # virtual tables per core
RPS = 9375                   # rows per slot (45*10000 / 48)
CJ = 37                      # rows per partition per chunk
CROWS = 128 * CJ             # 4736 rows per chunk
NCHUNK = 2                   # two full chunks per slot
RPS_PAD = NCHUNK * CROWS     # 9472 staged rows (97 zero-pad)
VCH = NCHUNK * CJ            # 74 score columns per slot
VPAD = VCH * 128             # 9472 flattened score-row length
NBLK = 8                     # gather blocks (Q7 groups)
BLK = 960                    # gathered values per block (dedup: 7680/slot)
GSLOT = NBLK * BLK           # 7680 gather slots per table-slot
SROW = BLK // 16             # 60 int16 idx entries per channel per slot
PAD_ROW = RPS                # 9375: a zero pad row

_COMPILED = {}


def _pos_of_local(local):
    """flattened score-row position for local row index (vectorized)."""
    c = local // CROWS
    r = local % CROWS
    return (c * CJ + r % CJ) * 128 + r // CJ


PAD_POS = int(_pos_of_local(np.int64(PAD_ROW)))   # zero score position


def _build_program():
    import concourse.bacc as bacc
    import concourse.tile as tile
    from concourse import mybir

    f32 = mybir.dt.float32
    bf16 = mybir.dt.bfloat16
    i16 = mybir.dt.int16

    nc = bacc.Bacc("TRN2", target_bir_lowering=False, debug=False,
                   num_devices=NCORES)

    tables_c = nc.dram_tensor("tables_c", [NSLOT, RPS_PAD, D], bf16,
                              kind="ExternalInput")
    ident_d = nc.dram_tensor("ident", [128, 128], f32, kind="ExternalInput")
    # idx staged host-side in the on-chip layout: one contiguous DMA (the
    # old [f p s]->[p f s] rearrange emitted 768 tiny descriptors, delaying
    # the first stream chunks behind it on the sync ring)
    idx_d = nc.dram_tensor("idx16", [128, NSLOT * SROW], i16,
                           kind="ExternalInput")
    out_d = nc.dram_tensor("out_part", [NSLOT, NBLK, BLK], f32,
                           kind="ExternalOutput")

    with tile.TileContext(nc) as tc:
        with (
            tc.tile_pool(name="const", bufs=1) as const_pool,
            tc.tile_pool(name="stream", bufs=3) as stream_pool,
            tc.tile_pool(name="tree", bufs=2) as tree_pool,
            tc.tile_pool(name="cols", bufs=3) as cols_pool,
            tc.tile_pool(name="row", bufs=2) as row_pool,
            tc.tile_pool(name="rep", bufs=2) as rep_pool,
            tc.tile_pool(name="pst", bufs=2, space="PSUM") as psum_t_pool,
        ):
            # one-time constants
            ident_t = const_pool.tile([128, 128], f32, tag="ident")
            nc.scalar.dma_start(ident_t[:], ident_d.ap())
            idx_t = const_pool.tile([128, NSLOT * SROW], i16, tag="idx")
            nc.scalar.dma_start(idx_t[:], idx_d.ap())
            gout_all = const_pool.tile([128, NSLOT * BLK], f32, tag="gout")

            # dummy 16-idx gather at t~0 forces the Q7 ap_gather library to
            # load while slot 0 streams (lazy load costs ~10us serial)
            zidx = const_pool.tile([128, 1], i16, tag="zidx")
            nc.vector.memset(zidx[:], 0)
            warm = const_pool.tile([128, 16], f32, tag="warm")
            winst = nc.gpsimd.ap_gather(
                out_ap=warm[:], in_ap=ident_t[:], idxs_ap=zidx[:],
                channels=128, num_elems=128, d=1, num_idxs=16)
            winst.verify = False

            tab_ap = tables_c.ap()  # [NSLOT, RPS_PAD, D]

            rep_hold = {}
            ptsb_hold = {}

            def phase2a_top(f, cols):
                # transpose -> PSUM evac; issued at the top of the NEXT
                # iteration so evac runs on DVE before that slot's trees.
                pt = psum_t_pool.tile([VCH, 128], f32, tag="pt")
                nc.tensor.transpose(pt[:], cols[:, :VCH], ident_t[:])
                ptsb = row_pool.tile([VCH, 128], f32, tag="ptsb")
                nc.vector.tensor_copy(ptsb[:], pt[:])
                ptsb_hold[f] = ptsb

            def phase2a_tail(f, split):
                # flatten row into rep partition 0, then log-replicate
                # p0 -> p{16,32,...,112}. split=True rides both rings: the
                # cross-ring dep chain idles the SDMA engines so the smalls
                # finish uncontended (used for slot 0 to start the Q7
                # gather chain ~50us earlier).
                ptsb = ptsb_hold.pop(f)
                ringA = nc.scalar if f % 2 == 0 else nc.sync
                ringB = nc.sync if f % 2 == 0 else nc.scalar
                if not split:
                    ringB = ringA
                rep = rep_pool.tile([128, VPAD], f32, tag="rep")
                ringA.dma_start(
                    rep[0:1, :].rearrange("o (c p) -> o c p", c=VCH), ptsb[:])
                repv = rep[:].rearrange("(a g) n -> a g n", g=16)
                ringB.dma_start(repv[1:2, 0], repv[0:1, 0])
                ringA.dma_start(repv[2:4, 0], repv[0:2, 0])
                ringB.dma_start(repv[4:8, 0], repv[0:4, 0])
                rep_hold[f] = rep

            def phase2b(f):
                # gpsimd runs ONLY ap_gathers; results accumulate in gout_all.
                rep = rep_hold.pop(f)
                inst = nc.gpsimd.ap_gather(
                    out_ap=gout_all[:, f * BLK:(f + 1) * BLK],
                    in_ap=rep[:, :VPAD],
                    idxs_ap=idx_t[:, f * SROW:(f + 1) * SROW],
                    channels=128,
                    num_elems=VPAD,
                    d=1,
                    num_idxs=BLK,
                )
                # skip the ucode's per-index bounds-verify pass (indices are
                # host-validated; they are always in [0, VPAD))
                inst.verify = False

            cols_hold = {}
            for f in range(NSLOT):
                if f >= 2:
                    phase2a_top(f - 1, cols_hold.pop(f - 1))
                cols = cols_pool.tile([128, VCH], f32, tag="cols")
                nc.vector.memset(cols[:], 0.0)
                # ---- phase 1: stream + tree-reduce (w pre-folded on host) --
                for c in range(NCHUNK):
                    st = stream_pool.tile([128, CJ * D], bf16, tag="st")
                    src_ap = tab_ap[f, c * CROWS:(c + 1) * CROWS, :].rearrange(
                        "(p j) d -> p j d", p=128)
                    ring = nc.sync if (f + c) % 2 == 0 else nc.scalar
                    ring.dma_start(st[:], src_ap)
                    # bf16 pairwise adds run in 2x packed mode; the final
                    # 16-wide tensor_reduce emits fp32 score columns.
                    stv = st[:].rearrange("p (j h d) -> p j h d", j=CJ, h=2)
                    s1 = tree_pool.tile([128, CJ * 128], bf16, tag="s1")
                    nc.vector.tensor_add(
                        s1[:].rearrange("p (j d) -> p j d", j=CJ),
                        stv[:, :, 0], stv[:, :, 1])
                    s1v = s1[:].rearrange("p (j h d) -> p j h d", j=CJ, h=2)
                    s2 = tree_pool.tile([128, CJ * 64], bf16, tag="s2")
                    nc.vector.tensor_add(
                        s2[:].rearrange("p (j d) -> p j d", j=CJ),
                        s1v[:, :, 0], s1v[:, :, 1])
                    s2v = s2[:].rearrange("p (j h d) -> p j h d", j=CJ, h=2)
                    s3 = tree_pool.tile([128, CJ * 32], bf16, tag="s3")
                    nc.vector.tensor_add(
                        s3[:].rearrange("p (j d) -> p j d", j=CJ),
                        s2v[:, :, 0], s2v[:, :, 1])
                    s3v = s3[:].rearrange("p (j h d) -> p j h d", j=CJ, h=2)
                    s4 = tree_pool.tile([128, CJ * 16], bf16, tag="s4")
                    nc.vector.tensor_add(
                        s4[:].rearrange("p (j d) -> p j d", j=CJ),
                        s3v[:, :, 0], s3v[:, :, 1])
                    nc.vector.tensor_reduce(
                        cols[:, c * CJ:(c + 1) * CJ],
                        s4[:].rearrange("p (j d) -> p j d", j=CJ),
                        axis=mybir.AxisListType.X,
                        op=mybir.AluOpType.add,
                    )

                if f == 0:
                    # slot 0 inline with ring-split smalls: fast chain start
                    phase2a_top(0, cols)
                    phase2a_tail(0, split=True)
                    phase2b(0)
                else:
                    cols_hold[f] = cols
                    if f >= 2:
                        phase2a_tail(f - 1, split=False)
                        phase2b(f - 1)
            phase2a_top(NSLOT - 1, cols_hold.pop(NSLOT - 1))
            phase2a_tail(NSLOT - 1, split=False)
            phase2b(NSLOT - 1)

            # single final output DMA: [8 group-base partitions, 6*960]
            nc.scalar.dma_start(
                out_d.ap().rearrange("f k n -> k f n"),
                gout_all[:].rearrange("(k g) (f n) -> k g f n", g=16, f=NSLOT
                                      )[:, 0])

    nc.compile()
    return nc


def _get_program():
    if "nc" not in _COMPILED:
        _COMPILED["nc"] = _build_program()
    return _COMPILED["nc"]


def kernel(src, tables, W1, b1, W2, b2, _trace=False, _trace_cores=None,
           _tmpdir=None):
    from concourse.bass_utils import run_bass_kernel_spmd

    src = np.asarray(src)
    tables = np.asarray(tables, dtype=np.float32)
    W1 = np.asarray(W1, dtype=np.float32)
    b1 = np.asarray(b1, dtype=np.float32)
    W2 = np.asarray(W2, dtype=np.float32)
    b2 = np.asarray(b2, dtype=np.float32)

    w = (W1 @ W2).reshape(D)                      # [256]
    c = float(b1 @ W2[:, 0] + b2[0])              # scalar per feature

    # fold w into the tables and cast to bf16; flatten to one row space
    flat = (tables.reshape(F * V, D) * w[None, :]).astype(ml_dtypes.bfloat16)

    RPC = NSLOT * RPS                             # 56250 rows per core
    src_i = np.asarray(src, dtype=np.int64)
    g = (np.arange(F, dtype=np.int64)[None, :] * V + src_i).ravel()   # [B*F]
    b_of = np.broadcast_to(
        np.arange(B, dtype=np.int32)[:, None], (B, F)).ravel()
    core_of = g // RPC
    slot_of = (g % RPC) // RPS
    local = (g % RPC) % RPS

    in_maps = []
    assembly = []   # per core per slot: (b_refs, inverse, spill)
    for core in range(NCORES):
        tc_arr = np.zeros((NSLOT, RPS_PAD, D), dtype=ml_dtypes.bfloat16)
        rows = flat[core * RPC:(core + 1) * RPC].reshape(NSLOT, RPS, D)
        tc_arr[:, :RPS, :] = rows

        idx16 = np.zeros((NSLOT, 128, SROW), dtype=np.int16)   # pre-transpose
        per_slot = []
        m_core = core_of == core
        for s in range(NSLOT):
            m = m_core & (slot_of == s)
            locs = local[m]
            bs = b_of[m]
            rows_u, inv = np.unique(locs, return_inverse=True)
            n_u = rows_u.shape[0]
            spill = None
            if n_u > GSLOT:
                # rows beyond device gather capacity: host-summed (rare)
                sp_rows = rows_u[GSLOT:]
                keep = inv < GSLOT
                spill = (sp_rows, bs[~keep], inv[~keep] - GSLOT)
                bs, inv = bs[keep], inv[keep]
                rows_u, n_u = rows_u[:GSLOT], GSLOT
            full = np.full(GSLOT, PAD_POS, dtype=np.int16)
            full[:n_u] = _pos_of_local(rows_u.astype(np.int64)).astype(np.int16)
            # idx16[s, 16k+p, t] = pos of distinct-row number BLK*k + 16t + p
            idx16[s] = (full.reshape(NBLK, SROW, 16)
                        .transpose(0, 2, 1)
                        .reshape(128, SROW))
            per_slot.append((bs, inv, spill))
        assembly.append(per_slot)
        in_maps.append({
            "tables_c": tc_arr,
            "ident": np.eye(128, dtype=np.float32),
            # [128, NSLOT*SROW]: the device's on-chip idx layout
            "idx16": np.ascontiguousarray(
                idx16.transpose(1, 0, 2).reshape(128, NSLOT * SROW)),
        })

    nc = _get_program()
    kw = {}
    if _trace:
        kw = {"trace": True, "trace_cores": _trace_cores or [0],
              "tmpdir": _tmpdir}
    res = run_bass_kernel_spmd(nc, in_maps, core_ids=list(range(NCORES)), **kw)
    _COMPILED["last_results"] = res

    total = np.zeros(B, dtype=np.float64)
    for core in range(NCORES):
        vals = res.results[core]["out_part"].reshape(NSLOT, GSLOT)
        for s in range(NSLOT):
            bs, inv, spill = assembly[core][s]
            np.add.at(total, bs, vals[s][inv].astype(np.float64))
            if spill is not None:
                sp_rows, sp_b, sp_inv = spill
                base = core * RPC + s * RPS
                sp_scores = flat[base + sp_rows].astype(np.float64).sum(axis=1)
                np.add.at(total, sp_b, sp_scores[sp_inv])
    total += F * c
    return total.astype(np.float32).reshape(B, 1)
